# revision 12
# baseline (speedup 1.0000x reference)
"""Trainium2 Bass kernel for the attention LSTM decoder (nn_Decoder).

Data-parallel over batch N=128 across 8 cores (NL=16 each); per core a
256-step teacher-forced decode. The per-step critical ring (PE gate
matmuls -> ACT tanh -> DVE fused pointwise -> attention -> softmax ->
context) is minimized:

  - LSTM pointwise via fused scalar_tensor_tensor ops (3 DVE ops/cell):
    cell buffer rows [t_i | t_f | t_g | S(=2c) | t_o] make every fusion a
    plain row slice:  uv = (cb[i,f]+1) * cb[g,S] -> [v, u];
    S' = 0.5u + v;  h' = (t_o+1)*tanh(S'/2)  (= 2h; consumer weight
    columns pre-halved, g-gate rows pre-doubled for the tanh trick).
    The o-gate tanh runs off the critical path.
  - softmax mask folded into the energy matmul: one constant triangular
    stationary x threshold-one-hot moving writes the -30000 step masks
    for all (slot, chunk) columns, including skipped chunks.
  - softmax denominators via 4 accumulating all-ones matmuls, produced
    already broadcast across partitions: one DVE reciprocal + one
    multiply, no gpsimd round trip.
  - PRE (embedding+bias) matmuls 2-packed at 64-row alignment (8 MMs),
    L2 bias via one select matmul, head bias folded into the ACT copy
    that stages 8 steps of logits per PSUM drain.
  - cross-step interleave: W_hh1/PRE matmuls of step t+2 are emitted
    under the pointwise chains of step t+1 to keep the PE fed; ragged
    encoder lengths prune attention chunks per slot (batch dealt to
    slots by sorted length so all cores share one SPMD program).
"""

import contextlib

import numpy as np
import ml_dtypes

import concourse.bacc as bacc
import concourse.bass as bass
import concourse.mybir as mybir
import concourse.tile as tile
from concourse.bass_utils import run_bass_kernel_spmd

T_ENC = 512
NB = 128
L_FULL = 256
KV = 128
VS = 128
EMB = 256
HID = 512
VOC = 34
NCORES = 8
NL = NB // NCORES          # batch per core = 16
TC = T_ENC // 128          # 4 t-chunks
P = 128

F32 = mybir.dt.float32
BF16 = mybir.dt.bfloat16
AF = mybir.ActivationFunctionType
OP = mybir.AluOpType
AX = mybir.AxisListType
BF16_NP = ml_dtypes.bfloat16

MASK_NEG = -30000.0
HEAD_BATCH = 8


def _gate_perm(h):
    # keep pytorch gate order [i, f, g, o]
    return np.arange(4 * h)


def _kernel_body(nc, tcx, io, L, reps=1, cnt=(TC,) * NL, plan=None):
    if plan is None:
        plan = tuple(('s', s, c) for s in range(NL) for c in range(cnt[s]))
    NCH = len(plan)
    with contextlib.ExitStack() as stack:
        const = stack.enter_context(tcx.tile_pool(name="const", bufs=1))
        work = stack.enter_context(tcx.tile_pool(name="work", bufs=2))

        # ---- constant SBUF tensors (loaded once) ----
        key_stat = const.tile([P, NCH, P], BF16, tag="key_stat")
        val_stat = const.tile([P, NCH, P], BF16, tag="val_stat")
        w1_stat = const.tile([P, 5, 16, P], BF16, tag="w1_stat")
        w2_stat = const.tile([P, 5, 4, P], BF16, tag="w2_stat")
        wout_stat = const.tile([P, 2, VOC], BF16, tag="wout_stat")
        b2all = const.tile([4, P], BF16, tag="b2all")
        sel4 = const.tile([4, 4, NL], BF16, tag="sel4")
        bout_col = const.tile([VOC, 1], F32, tag="bout_col")
        onehot3 = const.tile([P, L, 2, NL], BF16, tag="onehot3")
        pre_stat = const.tile([P, 8, P], BF16, tag="pre_stat")
        staging = const.tile([VOC, L, NL], F32, tag="staging")

        ones128 = const.tile([P, P], BF16, tag="ones128")
        tri_stat = const.tile([P, P], BF16, tag="tri_stat")
        thr_oh = const.tile([P, NL * TC], BF16, tag="thr_oh")

        # persistent cell buffers, rows [t_i | t_f | t_g | S(=2c) | t_o]:
        # the gate tanh lands around the S rows so the fused STTs below use
        # plain row slices: uv = (cb[i,f]+1) * cb[g,S] -> [v, u]
        cb1 = const.tile([P, 20, NL], F32, tag="cb1")
        cb2 = const.tile([P, 5, NL], F32, tag="cb2")
        h1d = const.tile([P, 4, NL], BF16, tag="h1d")
        h2d = const.tile([P, NL], BF16, tag="h2d")
        ctxT = const.tile([P, NL], BF16, tag="ctxT")

        # ---- prologue: DMA inputs ----
        nc.sync.dma_start(out=key_stat, in_=io["key_stat"].ap())
        nc.sync.dma_start(out=val_stat, in_=io["val_stat"].ap())
        nc.sync.dma_start(out=w1_stat, in_=io["w1_stat"].ap())
        nc.sync.dma_start(out=w2_stat, in_=io["w2_stat"].ap())
        nc.sync.dma_start(out=wout_stat, in_=io["wout_stat"].ap())
        nc.sync.dma_start(out=b2all, in_=io["b2all"].ap())
        nc.sync.dma_start(out=sel4, in_=io["sel4"].ap())
        nc.sync.dma_start(out=bout_col, in_=io["bout_col"].ap())
        nc.sync.dma_start(out=onehot3, in_=io["onehot3"].ap())
        nc.sync.dma_start(out=tri_stat, in_=io["tri_stat"].ap())
        nc.sync.dma_start(out=thr_oh, in_=io["thr_oh"].ap())

        nc.vector.memset(ones128, 1.0)
        nc.vector.memset(cb1, 0.0)
        nc.vector.memset(cb2, 0.0)
        nc.vector.memset(h1d, 0.0)
        nc.vector.memset(h2d, 0.0)
        nc.vector.memset(ctxT, 0.0)

        # ---- prologue: PRE = emb_ext.T @ w1e_rhs  -> (35, 2048) bf16 ----
        with tcx.tile_pool(name="prep", bufs=1) as prep, \
             tcx.tile_pool(name="prepp", bufs=1, space="PSUM") as prepp:
            emb_a = prep.tile([P, VOC + 1], F32, tag="emb_a")
            emb_b = prep.tile([P, VOC + 1], F32, tag="emb_b")
            emb_c = prep.tile([1, VOC + 1], F32, tag="emb_c")
            rhs_a = prep.tile([P, 2048], F32, tag="rhs_a")
            rhs_b = prep.tile([P, 2048], F32, tag="rhs_b")
            rhs_c = prep.tile([1, 2048], F32, tag="rhs_c")
            nc.sync.dma_start(out=emb_a, in_=io["emb_ext"].ap()[0:P, :])
            nc.sync.dma_start(out=emb_b, in_=io["emb_ext"].ap()[P:2 * P, :])
            nc.sync.dma_start(out=emb_c, in_=io["emb_ext"].ap()[2 * P:2 * P + 1, :])
            nc.sync.dma_start(out=rhs_a, in_=io["w1e_rhs"].ap()[0:P, :])
            nc.sync.dma_start(out=rhs_b, in_=io["w1e_rhs"].ap()[P:2 * P, :])
            nc.sync.dma_start(out=rhs_c, in_=io["w1e_rhs"].ap()[2 * P:2 * P + 1, :])
            nc.vector.memset(pre_stat, 0.0)
            for nf in range(4):
                pp = prepp.tile([VOC + 1, 512], F32, tag="prepsum")
                sl = slice(nf * 512, (nf + 1) * 512)
                nc.tensor.matmul(pp, emb_a, rhs_a[:, sl], start=True, stop=False)
                nc.tensor.matmul(pp, emb_b, rhs_b[:, sl], start=False, stop=False)
                nc.tensor.matmul(pp, emb_c, rhs_c[:, sl], start=False, stop=True)
                # gtile gt = nf*4 + j lands at pre_stat[64*(gt%2), gt//2, :]
                for j in range(4):
                    gt = nf * 4 + j
                    blk, row = gt // 2, gt % 2
                    nc.scalar.copy(
                        out=pre_stat[row * 64:row * 64 + VOC + 1, blk, :],
                        in_=pp[:, j * P:(j + 1) * P])

        pg1 = stack.enter_context(tcx.tile_pool(name="pg1", bufs=2, space="PSUM"))
        pg2 = stack.enter_context(tcx.tile_pool(name="pg2", bufs=1, space="PSUM"))
        pE = stack.enter_context(tcx.tile_pool(name="pE", bufs=1, space="PSUM"))
        psm = stack.enter_context(tcx.tile_pool(name="psm", bufs=1, space="PSUM"))
        pcx = stack.enter_context(tcx.tile_pool(name="pcx", bufs=1, space="PSUM"))
        ppd = stack.enter_context(tcx.tile_pool(name="ppd", bufs=1, space="PSUM"))
        pht = stack.enter_context(tcx.tile_pool(name="pht", bufs=1, space="PSUM"))

        att_list = [(s, c) for s in range(NL) for c in range(cnt[s])]

        ABL = set(__import__("os").environ.get("ABL", "").split(","))
        HEAT = int(__import__("os").environ.get("PE_HEAT", "0"))
        heat_tile = (pht.tile([P, P], F32, tag="heat", name="heat")
                     if HEAT else None)

        def heater(k):
            # dependency-free matmuls keep the PE p-state warm during the
            # pointwise chains (hardware re-throttles an idle PE)
            for _ in range(k):
                nc.tensor.matmul(heat_tile[:, 0:P], ones128,
                                 key_stat[:, 0, 0, :],
                                 start=True, stop=True, skip_group_check=True)
        ep = pE.tile([P, NL * TC], F32, tag="ep")

        g1ps = {}
        g2ps = {}
        pps = {}

        def emit_pre(t):
            g1p = pg1.tile([P, 16, NL], F32, tag="g1p", name="g1p")
            g1ps[t] = g1p
            for b in range(8):
                nc.tensor.matmul(g1p[:, 2 * b:2 * b + 2, :],
                                 pre_stat[:, b, :],
                                 onehot3[:, t, :, :],
                                 start=(b == 0), stop=False)

        def emit_h1mms(t):
            if "noh1" in ABL:
                return
            g1p = g1ps[t]
            for c in range(4):
                for g in range(16):
                    nc.tensor.matmul(g1p[:, g, :], w1_stat[:, 1 + c, g, :],
                                     h1d[:, c, :], start=False, stop=False)

        def emit_ctx_mms(t):
            g1p = g1ps[t]
            for g in ([15] if "noctx" in ABL else range(16)):
                nc.tensor.matmul(g1p[:, g, :], w1_stat[:, 0, g, :], ctxT,
                                 start=False, stop=(g == 15))

        def emit_l1_pointwise(t):
            g1p = g1ps.pop(t)
            if "nopw" in ABL:
                return
            # i,f,g tanh first (feeds uv); o tanh runs off the critical path
            nc.scalar.activation(cb1[:, 0:12, :], g1p[:, 0:12, :],
                                 AF.Tanh, scale=0.5)
            nc.scalar.activation(cb1[:, 16:20, :], g1p[:, 12:16, :],
                                 AF.Tanh, scale=0.5)
            uv1 = work.tile([P, 8, NL], F32, tag="uv1")
            th1 = work.tile([P, 4, NL], F32, tag="th1")
            # [v, u] = (cb[i, f] + 1) * cb[g, S]
            nc.vector.scalar_tensor_tensor(uv1, cb1[:, 0:8, :], 1.0,
                                           cb1[:, 8:16, :],
                                           op0=OP.add, op1=OP.mult)
            # S' = 0.5*u + v
            nc.vector.scalar_tensor_tensor(cb1[:, 12:16, :], uv1[:, 4:8, :],
                                           0.5, uv1[:, 0:4, :],
                                           op0=OP.mult, op1=OP.add)
            nc.scalar.activation(th1, cb1[:, 12:16, :], AF.Tanh, scale=0.5)
            nc.vector.scalar_tensor_tensor(h1d, cb1[:, 16:20, :], 1.0, th1,
                                           op0=OP.add, op1=OP.mult)

        def emit_g2(t):
            g2p = pg2.tile([P, 4, NL], F32, tag="g2p", name="g2p")
            g2ps[t] = g2p
            nc.tensor.matmul(g2p, b2all, sel4, start=True, stop=False)
            if "nog2" in ABL:
                nc.tensor.matmul(g2p[:, 3, :], w2_stat[:, 4, 3, :],
                                 h1d[:, 3, :], start=False, stop=True)
                return
            for g in range(4):
                nc.tensor.matmul(g2p[:, g, :], w2_stat[:, 0, g, :], h2d,
                                 start=False, stop=False)
            for c in range(4):
                for g in range(4):
                    nc.tensor.matmul(g2p[:, g, :], w2_stat[:, 1 + c, g, :],
                                     h1d[:, c, :],
                                     start=False, stop=(c == 3 and g == 3))

        def emit_l2_pointwise(t):
            g2p = g2ps.pop(t)
            if "nopw" in ABL:
                return
            nc.scalar.activation(cb2[:, 0:3, :], g2p[:, 0:3, :],
                                 AF.Tanh, scale=0.5)
            nc.scalar.activation(cb2[:, 4:5, :], g2p[:, 3:4, :],
                                 AF.Tanh, scale=0.5)
            uv2 = work.tile([P, 2, NL], F32, tag="uv2")
            th2 = work.tile([P, 1, NL], F32, tag="th2")
            nc.vector.scalar_tensor_tensor(uv2, cb2[:, 0:2, :], 1.0,
                                           cb2[:, 2:4, :],
                                           op0=OP.add, op1=OP.mult)
            nc.vector.scalar_tensor_tensor(cb2[:, 3:4, :], uv2[:, 1:2, :],
                                           0.5, uv2[:, 0:1, :],
                                           op0=OP.mult, op1=OP.add)
            nc.scalar.activation(th2, cb2[:, 3:4, :], AF.Tanh, scale=0.5)
            nc.vector.scalar_tensor_tensor(h2d, cb2[:, 4, :], 1.0,
                                           th2[:, 0, :],
                                           op0=OP.add, op1=OP.mult)

        HB = NL // 2

        def _att_aps(e, buf_pn, buf_ch):
            # (moving/out) APs for a plan entry against a [P, NL] tile
            # (buf_pn: h2d or cxp) and a flat [P, NL*TC] tile (buf_ch: ep
            # or expe)
            if e[0] == 's':
                _, s, c = e
                return buf_pn[:, s:s + 1], buf_ch[:, s * TC + c:s * TC + c + 1]
            _, a, ca, b, cb = e
            ia, ib = a * TC + ca, b * TC + cb
            return (buf_pn[:, a:b + 1:b - a] if b - a > 1 else buf_pn[:, a:b + 1],
                    buf_ch[:, ia:ib + 1:ib - ia])

        def emit_attention(t):
            # mask via one matmul: tri_stat[k, t] = MASK_NEG * [t >= k],
            # thr_oh[k, col] step-pattern masking invalid rows of each col
            nc.tensor.matmul(ep, tri_stat, thr_oh, start=True, stop=False)
            att_l = plan[-1:] if "noatt" in ABL else plan
            for i, e in enumerate(att_l):
                ci = plan.index(e) if "noatt" in ABL else i
                mov, out = _att_aps(e, h2d, ep)
                nc.tensor.matmul(out, key_stat[:, ci, :], mov,
                                 start=False,
                                 stop=(i == len(att_l) - 1))
            expe = work.tile([P, NL * TC], BF16, tag="expe")
            nc.scalar.activation(expe, ep, AF.Exp)

            # denominators, already broadcast across partitions: accumulate
            # ones128.T @ expe[:, :, tc] over the 4 chunks
            sums_b = psm.tile([P, NL], F32, tag="sums_b")
            sum_l = range(1) if "noatt" in ABL else range(TC)
            for tcc in sum_l:
                nc.tensor.matmul(sums_b, ones128, expe[:, tcc::TC],
                                 start=(tcc == 0), stop=(tcc == len(sum_l) - 1))
            cxp = pcx.tile([P, NL], F32, tag="cxp")
            cx_l = plan[-2:] if "noatt" in ABL else plan
            for i, e in enumerate(cx_l):
                ci = plan.index(e) if "noatt" in ABL else i
                mov, out = _att_aps(e, cxp, expe)
                nc.tensor.matmul(mov, val_stat[:, ci, :], out,
                                 start=(i == 0),
                                 stop=(i == len(cx_l) - 1))
            recip_b = work.tile([P, NL], F32, tag="recip_b")
            nc.vector.reciprocal(out=recip_b, in_=sums_b)
            nc.vector.tensor_mul(ctxT, cxp, recip_b)

        def emit_head(t):
            slot = t % HEAD_BATCH
            if slot == 0:
                pps[0] = ppd.tile([VOC, HEAD_BATCH, NL], F32, tag="pp", name="pp")
            pp = pps[0]
            nc.tensor.matmul(pp[:, slot, :], wout_stat[:, 0, :], h2d,
                             start=True, stop=False)
            nc.tensor.matmul(pp[:, slot, :], wout_stat[:, 1, :], ctxT,
                             start=False, stop=True)
            if slot == HEAD_BATCH - 1:
                nc.scalar.add(staging[:, t - slot:t + 1, :], pp, bout_col)

        rep_ctx = tcx.For_i(0, reps, 1) if reps > 1 else contextlib.nullcontext()
        with rep_ctx:
            # LSTM step 0 (ctx/h start at zero)
            emit_pre(0)
            emit_h1mms(0)
            emit_ctx_mms(0)
            emit_l1_pointwise(0)
            if L > 1:
                emit_pre(1)
            emit_g2(0)
            if L > 1:
                emit_h1mms(1)
            emit_l2_pointwise(0)

            for t in range(L):
                emit_attention(t)
                if HEAT:
                    heater(1)
                if t + 1 < L:
                    emit_ctx_mms(t + 1)
                emit_head(t)
                if t + 1 < L:
                    emit_l1_pointwise(t + 1)
                    if t + 2 < L:
                        emit_pre(t + 2)
                    if HEAT:
                        heater(1)
                    emit_g2(t + 1)
                    if t + 2 < L:
                        emit_h1mms(t + 2)
                    if HEAT:
                        heater(1)
                    emit_l2_pointwise(t + 1)

            if L % HEAD_BATCH:
                rem = L % HEAD_BATCH
                nc.scalar.copy(out=staging[:, L - rem:L, :],
                               in_=pps[0][:, 0:rem, :])

        # ---- epilogue ----
        nc.sync.dma_start(out=io["out"].ap(), in_=staging)


def build_module(L=L_FULL, reps=1, cnt=(TC,) * NL, plan=None):
    if plan is None:
        plan = tuple(('s', s, c) for s in range(NL) for c in range(cnt[s]))
    NCH = len(plan)
    nc = bacc.Bacc("TRN2", target_bir_lowering=False, debug=False)
    io = {}
    io["key_stat"] = nc.dram_tensor("key_stat", [P, NCH, P], BF16,
                                    kind="ExternalInput")
    io["val_stat"] = nc.dram_tensor("val_stat", [P, NCH, P], BF16,
                                    kind="ExternalInput")
    io["w1_stat"] = nc.dram_tensor("w1_stat", [P, 5, 16, P], BF16,
                                   kind="ExternalInput")
    io["w2_stat"] = nc.dram_tensor("w2_stat", [P, 5, 4, P], BF16,
                                   kind="ExternalInput")
    io["wout_stat"] = nc.dram_tensor("wout_stat", [P, 2, VOC], BF16,
                                     kind="ExternalInput")
    io["b2all"] = nc.dram_tensor("b2all", [4, P], BF16,
                                 kind="ExternalInput")
    io["sel4"] = nc.dram_tensor("sel4", [4, 4, NL], BF16,
                                kind="ExternalInput")
    io["bout_col"] = nc.dram_tensor("bout_col", [VOC, 1], F32,
                                    kind="ExternalInput")
    io["onehot3"] = nc.dram_tensor("onehot3", [P, L, 2, NL],
                                   BF16, kind="ExternalInput")
    io["tri_stat"] = nc.dram_tensor("tri_stat", [P, P], BF16,
                                    kind="ExternalInput")
    io["thr_oh"] = nc.dram_tensor("thr_oh", [P, NL * TC], BF16,
                                  kind="ExternalInput")
    io["emb_ext"] = nc.dram_tensor("emb_ext", [EMB + 1, VOC + 1], F32,
                                   kind="ExternalInput")
    io["w1e_rhs"] = nc.dram_tensor("w1e_rhs", [EMB + 1, 4 * HID], F32,
                                   kind="ExternalInput")
    io["out"] = nc.dram_tensor("out", [VOC, L, NL], F32,
                               kind="ExternalOutput")

    with tile.TileContext(nc) as tcx:
        _kernel_body(nc, tcx, io, L, reps=reps, cnt=cnt, plan=plan)
    nc.compile()
    return nc


def batch_assignment(lens):
    """Deal batch elements (sorted by length desc) round-robin to cores."""
    lens = np.asarray(lens).astype(np.int64)
    ranks = np.argsort(-lens, kind="stable")
    cnt = tuple(int(-(-lens[ranks[s * NCORES]] // 128)) for s in range(NL))
    return ranks, cnt


def make_plan(lens):
    """Attention chunk plan shared by all cores (SPMD template).

    Entries: ('s', s, c) one slot-chunk per stationary, or ('p', a, ca, b, cb)
    merging the partial tail chunks of slots a and b (a < b) into one
    stationary (softmax is permutation-invariant over t, so each slot's
    valid timesteps can be re-packed; per-core row offsets and masks are
    data, not program).
    """
    lens = np.asarray(lens).astype(np.int64)
    ranks, cnt = batch_assignment(lens)
    ra_max = []
    for s in range(NL):
        rs = [int(lens[ranks[s * NCORES + c]]) - 128 * (cnt[s] - 1)
              for c in range(NCORES)]
        ra_max.append(max(0, max(rs)))
    order = sorted(range(NL), key=lambda s: -ra_max[s])
    pairs, singles = [], []
    i, j = 0, NL - 1
    while i < j:
        a, b = order[i], order[j]
        if ra_max[a] + ra_max[b] <= P:
            pairs.append((a, b))
            i += 1
            j -= 1
        else:
            singles.append(a)
            i += 1
    if i == j:
        singles.append(order[i])
    plan = []
    for s in range(NL):
        for c in range(cnt[s] - 1):
            plan.append(('s', s, c))
    for s in sorted(singles):
        plan.append(('s', s, cnt[s] - 1))
    for a, b in pairs:
        a, b = min(a, b), max(a, b)
        plan.append(('p', a, cnt[a] - 1, b, cnt[b] - 1))
    return ranks, cnt, tuple(plan)


def make_in_maps(key, values, lens, text, emb, W_ih1, W_hh1, b_ih1, b_hh1,
                 W_ih2, W_hh2, b_ih2, b_hh2, W_out, b_out, L=L_FULL):
    key = np.asarray(key, np.float32)
    values = np.asarray(values, np.float32)
    lens = np.asarray(lens).astype(np.int64)
    text = np.asarray(text).astype(np.int64)
    emb = np.asarray(emb, np.float32)

    perm1 = _gate_perm(HID)
    W1 = np.concatenate([np.asarray(W_ih1, np.float32),
                         np.asarray(W_hh1, np.float32)], axis=1)[perm1]
    b1 = (np.asarray(b_ih1, np.float32) + np.asarray(b_hh1, np.float32))[perm1]
    W1 = W1.copy()
    b1 = b1.copy()
    W1[2 * HID:3 * HID] *= 2.0   # g-gate rows x2 (tanh trick)
    b1[2 * HID:3 * HID] *= 2.0
    W1[:, EMB + VS:] *= 0.5  # h1 stored as 2*h1

    perm2 = _gate_perm(KV)
    W2 = np.concatenate([np.asarray(W_ih2, np.float32),
                         np.asarray(W_hh2, np.float32)], axis=1)[perm2]
    b2 = (np.asarray(b_ih2, np.float32) + np.asarray(b_hh2, np.float32))[perm2]
    W2 = W2.copy()
    b2 = b2.copy()
    W2[2 * KV:3 * KV] *= 2.0
    b2[2 * KV:3 * KV] *= 2.0
    W2 *= 0.5                # both h1 and h2 inputs carry a 2x factor

    Wo = np.asarray(W_out, np.float32).copy()
    Wo[:, :KV] *= 0.5        # h2 stored as 2*h2
    bo = np.asarray(b_out, np.float32)

    # shared (same on all cores)
    emb_ext = np.zeros((EMB + 1, VOC + 1), np.float32)
    emb_ext[:EMB, :VOC] = emb.T
    emb_ext[EMB, VOC] = 1.0
    w1e_rhs = np.concatenate([W1[:, :EMB].T, b1[None, :]], axis=0)  # (257,2048)

    # w1_stat[k, c, g, m]: c=0 ctx (W1 cols 256:384), c=1..4 h1 (384:896)
    w1ct = W1[:, EMB:EMB + VS].T.reshape(P, 16, P)             # [k, g, m]
    w1h = W1[:, EMB + VS:].T.reshape(4, P, 16, P)              # [c, k, g, m]
    w1_stat = np.concatenate([w1ct[:, None], w1h.transpose(1, 0, 2, 3)],
                             axis=1).astype(BF16_NP)           # (128,5,16,128)

    w2h2 = W2[:, HID:].T.reshape(P, 4, P)                      # [k, g, m]
    w2h1 = W2[:, :HID].T.reshape(4, P, 4, P)                   # [c, k, g, m]
    w2_stat = np.concatenate([w2h2[:, None], w2h1.transpose(1, 0, 2, 3)],
                             axis=1).astype(BF16_NP)           # (128,5,4,128)
    b2all = b2.reshape(4, P).astype(BF16_NP)
    sel4 = np.zeros((4, 4, NL), np.float32)
    for g in range(4):
        sel4[g, g, :] = 1.0
    sel4 = sel4.astype(BF16_NP)

    wout_stat = np.stack([Wo[:, :KV].T, Wo[:, KV:].T], axis=1).astype(BF16_NP)
    bout_col = bo.reshape(VOC, 1).astype(np.float32)

    tri = MASK_NEG * (np.arange(P)[None, :] >= np.arange(P)[:, None])
    tri_stat = tri.astype(BF16_NP)                             # [k, t]

    shared = dict(emb_ext=emb_ext, w1e_rhs=w1e_rhs, w1_stat=w1_stat,
                  w2_stat=w2_stat, b2all=b2all, sel4=sel4,
                  wout_stat=wout_stat, bout_col=bout_col, tri_stat=tri_stat)

    ranks, cnt, plan = make_plan(lens)
    NCH = len(plan)
    in_maps = []
    key_h = key * 0.5        # energy uses h2 stored as 2*h2
    for c in range(NCORES):
        B = ranks[c::NCORES]
        kh = key_h[:, B, :]                                    # (T, 16, KV)
        vv = values[:, B, :]                                   # (T, 16, VS)
        lensB = lens[B]
        # packed stationaries: key_pack[k, ci, t], val_pack[t, ci, v]
        key_pack = np.zeros((KV, NCH, P), np.float32)
        val_pack = np.zeros((P, NCH, VS), np.float32)
        # thr masks, step-pattern per (slot, chunk) column
        thr = np.clip(lensB[:, None] - 128 * np.arange(TC)[None, :], 0, P)
        th_oh = np.zeros((P + 1, NL, TC), np.float32)
        nn2, tc2 = np.meshgrid(np.arange(NL), np.arange(TC), indexing="ij")
        th_oh[thr.reshape(-1), nn2.reshape(-1), tc2.reshape(-1)] = 1.0

        def tail_rows(s):
            return int(np.clip(lensB[s] - 128 * (cnt[s] - 1), 0, P))

        for ci, e in enumerate(plan):
            if e[0] == 's':
                _, s, cc = e
                key_pack[:, ci, :] = kh[128 * cc:128 * (cc + 1), s, :].T
                val_pack[:, ci, :] = vv[128 * cc:128 * (cc + 1), s, :]
            else:
                _, a, ca, b, cb = e
                ra, rb = tail_rows(a), tail_rows(b)
                if ra:
                    key_pack[:, ci, :ra] = kh[128 * ca:128 * ca + ra, a, :].T
                    val_pack[:ra, ci, :] = vv[128 * ca:128 * ca + ra, a, :]
                if rb:
                    key_pack[:, ci, ra:ra + rb] = \
                        kh[128 * cb:128 * cb + rb, b, :].T
                    val_pack[ra:ra + rb, ci, :] = \
                        vv[128 * cb:128 * cb + rb, b, :]
                # col a: mask t >= ra (one-hot at ra, as already built)
                # col b: mask t < ra and t >= ra+rb: steps +1@0 -1@ra +1@ra+rb
                vec = np.zeros(P + 1, np.float32)
                vec[0] += 1.0
                vec[min(ra, P)] -= 1.0
                vec[min(ra + rb, P)] += 1.0
                th_oh[:P, b, cb] = vec[:P]
                th_oh[P, b, cb] = 0.0
                vec2 = np.zeros(P + 1, np.float32)
                vec2[min(ra, P)] += 1.0
                th_oh[:P, a, ca] = vec2[:P]
                th_oh[P, a, ca] = 0.0
        th_oh = th_oh[:P].reshape(P, NL * TC)

        oh = np.zeros((VOC + 1, L, NL), np.float32)
        txt = text[B, :L]                                      # (16, L)
        nn_idx, tt_idx = np.meshgrid(np.arange(NL), np.arange(L),
                                     indexing="ij")
        oh[txt.reshape(-1), tt_idx.reshape(-1), nn_idx.reshape(-1)] = 1.0
        oh[VOC, :, :] = 1.0
        oh3 = np.zeros((P, L, 2, NL), np.float32)              # block-diag
        for j in range(2):
            oh3[j * 64:j * 64 + VOC + 1, :, j, :] = oh
        in_maps.append(dict(shared,
                            key_stat=key_pack.astype(BF16_NP),
                            val_stat=val_pack.astype(BF16_NP),
                            onehot3=oh3.astype(BF16_NP),
                            thr_oh=th_oh.astype(BF16_NP)))
    return in_maps


_MODULE_CACHE = {}


def kernel(**inputs):
    L = int(np.asarray(inputs["text"]).shape[1])
    ranks, cnt, plan = make_plan(inputs["lens"])
    key_c = (L, cnt, plan)
    if key_c not in _MODULE_CACHE:
        _MODULE_CACHE[key_c] = build_module(L, cnt=cnt, plan=plan)
    nc = _MODULE_CACHE[key_c]
    in_maps = make_in_maps(**inputs, L=L)
    res = run_bass_kernel_spmd(nc, in_maps, core_ids=list(range(NCORES)))
    out = np.zeros((NB, L, VOC), np.float32)
    for c in range(NCORES):
        o = res.results[c]["out"]                              # (34, L, 16)
        for s in range(NL):
            out[ranks[s * NCORES + c]] = o[:, :, s].T
    return out



# revision 13
# speedup vs baseline: 1.6351x; 1.6351x over previous
"""Trainium2 Bass kernel for the attention LSTM decoder (nn_Decoder).

Data-parallel over batch N=128 across 8 cores (NL=16 each); per core a
256-step teacher-forced decode. The per-step critical ring (PE gate
matmuls -> ACT tanh -> DVE fused pointwise -> attention -> softmax ->
context) is minimized:

  - LSTM pointwise via fused scalar_tensor_tensor ops (3 DVE ops/cell):
    cell buffer rows [t_i | t_f | t_g | S(=2c) | t_o] make every fusion a
    plain row slice:  uv = (cb[i,f]+1) * cb[g,S] -> [v, u];
    S' = 0.5u + v;  h' = (t_o+1)*tanh(S'/2)  (= 2h; consumer weight
    columns pre-halved, g-gate rows pre-doubled for the tanh trick).
    The o-gate tanh runs off the critical path.
  - softmax mask folded into the energy matmul: one constant triangular
    stationary x threshold-one-hot moving writes the -30000 step masks
    for all (slot, chunk) columns, including skipped chunks.
  - softmax denominators via 4 accumulating all-ones matmuls, produced
    already broadcast across partitions: one DVE reciprocal + one
    multiply, no gpsimd round trip.
  - PRE (embedding+bias) matmuls 2-packed at 64-row alignment (8 MMs),
    L2 bias via one select matmul, head bias folded into the ACT copy
    that stages 8 steps of logits per PSUM drain.
  - cross-step interleave: W_hh1/PRE matmuls of step t+2 are emitted
    under the pointwise chains of step t+1 to keep the PE fed; ragged
    encoder lengths prune attention chunks per slot (batch dealt to
    slots by sorted length so all cores share one SPMD program).
"""

import contextlib

import numpy as np
import ml_dtypes

import concourse.bacc as bacc
import concourse.bass as bass
import concourse.mybir as mybir
import concourse.tile as tile
from concourse.bass_utils import run_bass_kernel_spmd

T_ENC = 512
NB = 128
L_FULL = 256
KV = 128
VS = 128
EMB = 256
HID = 512
VOC = 34
NCORES = 8
NL = NB // NCORES          # batch per core = 16
TC = T_ENC // 128          # 4 t-chunks
P = 128

F32 = mybir.dt.float32
BF16 = mybir.dt.bfloat16
AF = mybir.ActivationFunctionType
OP = mybir.AluOpType
AX = mybir.AxisListType
BF16_NP = ml_dtypes.bfloat16

MASK_NEG = -30000.0
HEAD_BATCH = 8


def _gate_perm(h):
    # keep pytorch gate order [i, f, g, o]
    return np.arange(4 * h)


def _kernel_body(nc, tcx, io, L, reps=1, cnt=(TC,) * NL, plan=None):
    if plan is None:
        plan = tuple(('s', s, c) for s in range(NL) for c in range(cnt[s]))
    NCH = len(plan)
    with contextlib.ExitStack() as stack:
        const = stack.enter_context(tcx.tile_pool(name="const", bufs=1))
        work = stack.enter_context(tcx.tile_pool(name="work", bufs=2))

        # ---- constant SBUF tensors (loaded once) ----
        key_stat = const.tile([P, NCH, P], BF16, tag="key_stat")
        val_stat = const.tile([P, NCH, P], BF16, tag="val_stat")
        w1_stat = const.tile([P, 5, 16, P], BF16, tag="w1_stat")
        w2_stat = const.tile([P, 5, 4, P], BF16, tag="w2_stat")
        wout_stat = const.tile([P, 2, VOC], BF16, tag="wout_stat")
        b2all = const.tile([4, P], BF16, tag="b2all")
        sel4 = const.tile([4, 4, NL], BF16, tag="sel4")
        bout_col = const.tile([VOC, 1], F32, tag="bout_col")
        onehot3 = const.tile([P, L, 2, NL], BF16, tag="onehot3")
        pre_stat = const.tile([P, 8, P], BF16, tag="pre_stat")
        staging = const.tile([VOC, L, NL], F32, tag="staging")

        ones128 = const.tile([P, P], BF16, tag="ones128")
        tri_stat = const.tile([P, P], BF16, tag="tri_stat")
        thr_oh = const.tile([P, NL * TC], BF16, tag="thr_oh")

        # persistent cell buffers, rows [t_i | t_f | t_g | S(=2c) | t_o]:
        # the gate tanh lands around the S rows so the fused STTs below use
        # plain row slices: uv = (cb[i,f]+1) * cb[g,S] -> [v, u]
        cb1 = const.tile([P, 20, NL], F32, tag="cb1")
        cb2 = const.tile([P, 5, NL], F32, tag="cb2")
        h1d = const.tile([P, 4, NL], BF16, tag="h1d")
        h2d = const.tile([P, NL], BF16, tag="h2d")
        ctxT = const.tile([P, NL], BF16, tag="ctxT")

        # ---- prologue: DMA inputs ----
        nc.sync.dma_start(out=key_stat, in_=io["key_stat"].ap())
        nc.sync.dma_start(out=val_stat, in_=io["val_stat"].ap())
        nc.sync.dma_start(out=w1_stat, in_=io["w1_stat"].ap())
        nc.sync.dma_start(out=w2_stat, in_=io["w2_stat"].ap())
        nc.sync.dma_start(out=wout_stat, in_=io["wout_stat"].ap())
        nc.sync.dma_start(out=b2all, in_=io["b2all"].ap())
        nc.sync.dma_start(out=sel4, in_=io["sel4"].ap())
        nc.sync.dma_start(out=bout_col, in_=io["bout_col"].ap())
        nc.sync.dma_start(out=onehot3, in_=io["onehot3"].ap())
        nc.sync.dma_start(out=tri_stat, in_=io["tri_stat"].ap())
        nc.sync.dma_start(out=thr_oh, in_=io["thr_oh"].ap())

        nc.vector.memset(ones128, 1.0)
        nc.vector.memset(cb1, 0.0)
        nc.vector.memset(cb2, 0.0)
        nc.vector.memset(h1d, 0.0)
        nc.vector.memset(h2d, 0.0)
        nc.vector.memset(ctxT, 0.0)

        # ---- prologue: PRE = emb_ext.T @ w1e_rhs  -> (35, 2048) bf16 ----
        with tcx.tile_pool(name="prep", bufs=1) as prep, \
             tcx.tile_pool(name="prepp", bufs=1, space="PSUM") as prepp:
            emb_a = prep.tile([P, VOC + 1], F32, tag="emb_a")
            emb_b = prep.tile([P, VOC + 1], F32, tag="emb_b")
            emb_c = prep.tile([1, VOC + 1], F32, tag="emb_c")
            rhs_a = prep.tile([P, 2048], F32, tag="rhs_a")
            rhs_b = prep.tile([P, 2048], F32, tag="rhs_b")
            rhs_c = prep.tile([1, 2048], F32, tag="rhs_c")
            nc.sync.dma_start(out=emb_a, in_=io["emb_ext"].ap()[0:P, :])
            nc.sync.dma_start(out=emb_b, in_=io["emb_ext"].ap()[P:2 * P, :])
            nc.sync.dma_start(out=emb_c, in_=io["emb_ext"].ap()[2 * P:2 * P + 1, :])
            nc.sync.dma_start(out=rhs_a, in_=io["w1e_rhs"].ap()[0:P, :])
            nc.sync.dma_start(out=rhs_b, in_=io["w1e_rhs"].ap()[P:2 * P, :])
            nc.sync.dma_start(out=rhs_c, in_=io["w1e_rhs"].ap()[2 * P:2 * P + 1, :])
            nc.vector.memset(pre_stat, 0.0)
            for nf in range(4):
                pp = prepp.tile([VOC + 1, 512], F32, tag="prepsum")
                sl = slice(nf * 512, (nf + 1) * 512)
                nc.tensor.matmul(pp, emb_a, rhs_a[:, sl], start=True, stop=False)
                nc.tensor.matmul(pp, emb_b, rhs_b[:, sl], start=False, stop=False)
                nc.tensor.matmul(pp, emb_c, rhs_c[:, sl], start=False, stop=True)
                # gtile gt = nf*4 + j lands at pre_stat[64*(gt%2), gt//2, :]
                for j in range(4):
                    gt = nf * 4 + j
                    blk, row = gt // 2, gt % 2
                    nc.scalar.copy(
                        out=pre_stat[row * 64:row * 64 + VOC + 1, blk, :],
                        in_=pp[:, j * P:(j + 1) * P])

        pg1 = stack.enter_context(tcx.tile_pool(name="pg1", bufs=2, space="PSUM"))
        pg2 = stack.enter_context(tcx.tile_pool(name="pg2", bufs=1, space="PSUM"))
        pE = stack.enter_context(tcx.tile_pool(name="pE", bufs=1, space="PSUM"))
        psm = stack.enter_context(tcx.tile_pool(name="psm", bufs=1, space="PSUM"))
        pcx = stack.enter_context(tcx.tile_pool(name="pcx", bufs=1, space="PSUM"))
        ppd = stack.enter_context(tcx.tile_pool(name="ppd", bufs=1, space="PSUM"))
        pht = stack.enter_context(tcx.tile_pool(name="pht", bufs=1, space="PSUM"))

        att_list = [(s, c) for s in range(NL) for c in range(cnt[s])]

        ABL = set(__import__("os").environ.get("ABL", "").split(","))
        HEAT = int(__import__("os").environ.get("PE_HEAT", "0"))
        heat_tile = (pht.tile([P, P], F32, tag="heat", name="heat")
                     if HEAT else None)

        def heater(k):
            # dependency-free matmuls keep the PE p-state warm during the
            # pointwise chains (hardware re-throttles an idle PE)
            for _ in range(k):
                nc.tensor.matmul(heat_tile[:, 0:P], ones128,
                                 key_stat[:, 0, 0, :],
                                 start=True, stop=True, skip_group_check=True)
        ep = pE.tile([P, NL * TC], F32, tag="ep")

        g1ps = {}
        g2ps = {}
        pps = {}

        def emit_pre(t):
            g1p = pg1.tile([P, 16, NL], F32, tag="g1p", name="g1p")
            g1ps[t] = g1p
            for b in range(8):
                nc.tensor.matmul(g1p[:, 2 * b:2 * b + 2, :],
                                 pre_stat[:, b, :],
                                 onehot3[:, t, :, :],
                                 start=(b == 0), stop=False)

        def emit_h1mms(t):
            if "noh1" in ABL:
                return
            g1p = g1ps[t]
            for c in range(4):
                for g in range(16):
                    nc.tensor.matmul(g1p[:, g, :], w1_stat[:, 1 + c, g, :],
                                     h1d[:, c, :], start=False, stop=False)

        def emit_ctx_mms(t):
            g1p = g1ps[t]
            for g in ([15] if "noctx" in ABL else range(16)):
                nc.tensor.matmul(g1p[:, g, :], w1_stat[:, 0, g, :], ctxT,
                                 start=False, stop=(g == 15))

        def emit_l1_pointwise(t):
            g1p = g1ps.pop(t)
            if "nopw" in ABL:
                return
            # i,f,g tanh first (feeds uv); o tanh runs off the critical path
            nc.scalar.activation(cb1[:, 0:12, :], g1p[:, 0:12, :],
                                 AF.Tanh, scale=0.5)
            nc.scalar.activation(cb1[:, 16:20, :], g1p[:, 12:16, :],
                                 AF.Tanh, scale=0.5)
            uv1 = work.tile([P, 8, NL], F32, tag="uv1")
            th1 = work.tile([P, 4, NL], F32, tag="th1")
            # [v, u] = (cb[i, f] + 1) * cb[g, S]
            nc.vector.scalar_tensor_tensor(uv1, cb1[:, 0:8, :], 1.0,
                                           cb1[:, 8:16, :],
                                           op0=OP.add, op1=OP.mult)
            # S' = 0.5*u + v
            nc.vector.scalar_tensor_tensor(cb1[:, 12:16, :], uv1[:, 4:8, :],
                                           0.5, uv1[:, 0:4, :],
                                           op0=OP.mult, op1=OP.add)
            nc.scalar.activation(th1, cb1[:, 12:16, :], AF.Tanh, scale=0.5)
            nc.vector.scalar_tensor_tensor(h1d, cb1[:, 16:20, :], 1.0, th1,
                                           op0=OP.add, op1=OP.mult)

        def emit_g2(t):
            g2p = pg2.tile([P, 4, NL], F32, tag="g2p", name="g2p")
            g2ps[t] = g2p
            nc.tensor.matmul(g2p, b2all, sel4, start=True, stop=False)
            if "nog2" in ABL:
                nc.tensor.matmul(g2p[:, 3, :], w2_stat[:, 4, 3, :],
                                 h1d[:, 3, :], start=False, stop=True)
                return
            for g in range(4):
                nc.tensor.matmul(g2p[:, g, :], w2_stat[:, 0, g, :], h2d,
                                 start=False, stop=False)
            for c in range(4):
                for g in range(4):
                    nc.tensor.matmul(g2p[:, g, :], w2_stat[:, 1 + c, g, :],
                                     h1d[:, c, :],
                                     start=False, stop=(c == 3 and g == 3))

        def emit_l2_pointwise(t):
            g2p = g2ps.pop(t)
            if "nopw" in ABL:
                return
            nc.scalar.activation(cb2[:, 0:3, :], g2p[:, 0:3, :],
                                 AF.Tanh, scale=0.5)
            nc.scalar.activation(cb2[:, 4:5, :], g2p[:, 3:4, :],
                                 AF.Tanh, scale=0.5)
            uv2 = work.tile([P, 2, NL], F32, tag="uv2")
            th2 = work.tile([P, 1, NL], F32, tag="th2")
            nc.vector.scalar_tensor_tensor(uv2, cb2[:, 0:2, :], 1.0,
                                           cb2[:, 2:4, :],
                                           op0=OP.add, op1=OP.mult)
            nc.vector.scalar_tensor_tensor(cb2[:, 3:4, :], uv2[:, 1:2, :],
                                           0.5, uv2[:, 0:1, :],
                                           op0=OP.mult, op1=OP.add)
            nc.scalar.activation(th2, cb2[:, 3:4, :], AF.Tanh, scale=0.5)
            nc.vector.scalar_tensor_tensor(h2d, cb2[:, 4, :], 1.0,
                                           th2[:, 0, :],
                                           op0=OP.add, op1=OP.mult)

        HB = NL // 2

        def _att_aps(e, buf_pn, buf_ch):
            # (moving/out) APs for a plan entry against a [P, NL] tile
            # (buf_pn: h2d or cxp) and a flat [P, NL*TC] tile (buf_ch: ep
            # or expe)
            if e[0] == 's':
                _, s, c = e
                return buf_pn[:, s:s + 1], buf_ch[:, s * TC + c:s * TC + c + 1]
            _, a, ca, b, cb = e
            ia, ib = a * TC + ca, b * TC + cb
            return (buf_pn[:, a:b + 1:b - a] if b - a > 1 else buf_pn[:, a:b + 1],
                    buf_ch[:, ia:ib + 1:ib - ia])

        def emit_attention(t):
            # mask via one matmul: tri_stat[k, t] = MASK_NEG * [t >= k],
            # thr_oh[k, col] step-pattern masking invalid rows of each col
            nc.tensor.matmul(ep, tri_stat, thr_oh, start=True, stop=False)
            att_l = plan[-1:] if "noatt" in ABL else plan
            for i, e in enumerate(att_l):
                ci = plan.index(e) if "noatt" in ABL else i
                mov, out = _att_aps(e, h2d, ep)
                nc.tensor.matmul(out, key_stat[:, ci, :], mov,
                                 start=False,
                                 stop=(i == len(att_l) - 1))
            expe = work.tile([P, NL * TC], BF16, tag="expe")
            nc.scalar.activation(expe, ep, AF.Exp)

            # denominators, already broadcast across partitions: accumulate
            # ones128.T @ expe[:, :, tc] over the 4 chunks
            sums_b = psm.tile([P, NL], F32, tag="sums_b")
            sum_l = range(1) if "noatt" in ABL else range(TC)
            for tcc in sum_l:
                nc.tensor.matmul(sums_b, ones128, expe[:, tcc::TC],
                                 start=(tcc == 0), stop=(tcc == len(sum_l) - 1))
            cxp = pcx.tile([P, NL], F32, tag="cxp")
            cx_l = plan[-2:] if "noatt" in ABL else plan
            for i, e in enumerate(cx_l):
                ci = plan.index(e) if "noatt" in ABL else i
                mov, out = _att_aps(e, cxp, expe)
                nc.tensor.matmul(mov, val_stat[:, ci, :], out,
                                 start=(i == 0),
                                 stop=(i == len(cx_l) - 1))
            recip_b = work.tile([P, NL], F32, tag="recip_b")
            nc.vector.reciprocal(out=recip_b, in_=sums_b)
            nc.vector.tensor_mul(ctxT, cxp, recip_b)

        def emit_head(t):
            slot = t % HEAD_BATCH
            if slot == 0:
                pps[0] = ppd.tile([VOC, HEAD_BATCH, NL], F32, tag="pp", name="pp")
            pp = pps[0]
            nc.tensor.matmul(pp[:, slot, :], wout_stat[:, 0, :], h2d,
                             start=True, stop=False)
            nc.tensor.matmul(pp[:, slot, :], wout_stat[:, 1, :], ctxT,
                             start=False, stop=True)
            if slot == HEAD_BATCH - 1:
                nc.scalar.add(staging[:, t - slot:t + 1, :], pp, bout_col)

        rep_ctx = tcx.For_i(0, reps, 1) if reps > 1 else contextlib.nullcontext()
        with rep_ctx:
            # LSTM step 0 (ctx/h start at zero)
            emit_pre(0)
            emit_h1mms(0)
            emit_ctx_mms(0)
            emit_l1_pointwise(0)
            if L > 1:
                emit_pre(1)
            emit_g2(0)
            if L > 1:
                emit_h1mms(1)
            emit_l2_pointwise(0)

            for t in range(L):
                emit_attention(t)
                if HEAT:
                    heater(1)
                if t + 1 < L:
                    emit_ctx_mms(t + 1)
                emit_head(t)
                if t + 1 < L:
                    emit_l1_pointwise(t + 1)
                    if t + 2 < L:
                        emit_pre(t + 2)
                    if HEAT:
                        heater(1)
                    emit_g2(t + 1)
                    if t + 2 < L:
                        emit_h1mms(t + 2)
                    if HEAT:
                        heater(1)
                    emit_l2_pointwise(t + 1)

            if L % HEAD_BATCH:
                rem = L % HEAD_BATCH
                nc.scalar.copy(out=staging[:, L - rem:L, :],
                               in_=pps[0][:, 0:rem, :])

        # ---- epilogue ----
        nc.sync.dma_start(out=io["out"].ap(), in_=staging)


def build_module(L=L_FULL, reps=1, cnt=(TC,) * NL, plan=None):
    if plan is None:
        plan = tuple(('s', s, c) for s in range(NL) for c in range(cnt[s]))
    NCH = len(plan)
    nc = bacc.Bacc("TRN2", target_bir_lowering=False, debug=False)
    io = {}
    io["key_stat"] = nc.dram_tensor("key_stat", [P, NCH, P], BF16,
                                    kind="ExternalInput")
    io["val_stat"] = nc.dram_tensor("val_stat", [P, NCH, P], BF16,
                                    kind="ExternalInput")
    io["w1_stat"] = nc.dram_tensor("w1_stat", [P, 5, 16, P], BF16,
                                   kind="ExternalInput")
    io["w2_stat"] = nc.dram_tensor("w2_stat", [P, 5, 4, P], BF16,
                                   kind="ExternalInput")
    io["wout_stat"] = nc.dram_tensor("wout_stat", [P, 2, VOC], BF16,
                                     kind="ExternalInput")
    io["b2all"] = nc.dram_tensor("b2all", [4, P], BF16,
                                 kind="ExternalInput")
    io["sel4"] = nc.dram_tensor("sel4", [4, 4, NL], BF16,
                                kind="ExternalInput")
    io["bout_col"] = nc.dram_tensor("bout_col", [VOC, 1], F32,
                                    kind="ExternalInput")
    io["onehot3"] = nc.dram_tensor("onehot3", [P, L, 2, NL],
                                   BF16, kind="ExternalInput")
    io["tri_stat"] = nc.dram_tensor("tri_stat", [P, P], BF16,
                                    kind="ExternalInput")
    io["thr_oh"] = nc.dram_tensor("thr_oh", [P, NL * TC], BF16,
                                  kind="ExternalInput")
    io["emb_ext"] = nc.dram_tensor("emb_ext", [EMB + 1, VOC + 1], F32,
                                   kind="ExternalInput")
    io["w1e_rhs"] = nc.dram_tensor("w1e_rhs", [EMB + 1, 4 * HID], F32,
                                   kind="ExternalInput")
    io["out"] = nc.dram_tensor("out", [VOC, L, NL], F32,
                               kind="ExternalOutput")

    with tile.TileContext(nc) as tcx:
        _kernel_body(nc, tcx, io, L, reps=reps, cnt=cnt, plan=plan)
    nc.compile()
    return nc


def batch_assignment(lens):
    """Deal batch elements (sorted by length desc) round-robin to cores."""
    lens = np.asarray(lens).astype(np.int64)
    ranks = np.argsort(-lens, kind="stable")
    cnt = tuple(int(-(-lens[ranks[s * NCORES]] // 128)) for s in range(NL))
    return ranks, cnt


def make_plan(lens):
    """Attention chunk plan shared by all cores (SPMD template).

    Entries: ('s', s, c) one slot-chunk per stationary, or ('p', a, ca, b, cb)
    merging the partial tail chunks of slots a and b (a < b) into one
    stationary (softmax is permutation-invariant over t, so each slot's
    valid timesteps can be re-packed; per-core row offsets and masks are
    data, not program).
    """
    lens = np.asarray(lens).astype(np.int64)
    ranks, cnt = batch_assignment(lens)
    ra_max = []
    for s in range(NL):
        rs = [int(lens[ranks[s * NCORES + c]]) - 128 * (cnt[s] - 1)
              for c in range(NCORES)]
        ra_max.append(max(0, max(rs)))
    order = sorted(range(NL), key=lambda s: -ra_max[s])
    pairs, singles = [], []
    i, j = 0, NL - 1
    if __import__("os").environ.get("NOPAIR"):
        i = NL
        singles = list(range(NL))
    while i < j:
        a, b = order[i], order[j]
        if ra_max[a] + ra_max[b] <= P:
            pairs.append((a, b))
            i += 1
            j -= 1
        else:
            singles.append(a)
            i += 1
    if i == j:
        singles.append(order[i])
    plan = []
    for s in range(NL):
        for c in range(cnt[s] - 1):
            plan.append(('s', s, c))
    for s in sorted(singles):
        plan.append(('s', s, cnt[s] - 1))
    for a, b in pairs:
        a, b = min(a, b), max(a, b)
        plan.append(('p', a, cnt[a] - 1, b, cnt[b] - 1))
    return ranks, cnt, tuple(plan)


def make_in_maps(key, values, lens, text, emb, W_ih1, W_hh1, b_ih1, b_hh1,
                 W_ih2, W_hh2, b_ih2, b_hh2, W_out, b_out, L=L_FULL):
    key = np.asarray(key, np.float32)
    values = np.asarray(values, np.float32)
    lens = np.asarray(lens).astype(np.int64)
    text = np.asarray(text).astype(np.int64)
    emb = np.asarray(emb, np.float32)

    perm1 = _gate_perm(HID)
    W1 = np.concatenate([np.asarray(W_ih1, np.float32),
                         np.asarray(W_hh1, np.float32)], axis=1)[perm1]
    b1 = (np.asarray(b_ih1, np.float32) + np.asarray(b_hh1, np.float32))[perm1]
    W1 = W1.copy()
    b1 = b1.copy()
    W1[2 * HID:3 * HID] *= 2.0   # g-gate rows x2 (tanh trick)
    b1[2 * HID:3 * HID] *= 2.0
    W1[:, EMB + VS:] *= 0.5  # h1 stored as 2*h1

    perm2 = _gate_perm(KV)
    W2 = np.concatenate([np.asarray(W_ih2, np.float32),
                         np.asarray(W_hh2, np.float32)], axis=1)[perm2]
    b2 = (np.asarray(b_ih2, np.float32) + np.asarray(b_hh2, np.float32))[perm2]
    W2 = W2.copy()
    b2 = b2.copy()
    W2[2 * KV:3 * KV] *= 2.0
    b2[2 * KV:3 * KV] *= 2.0
    W2 *= 0.5                # both h1 and h2 inputs carry a 2x factor

    Wo = np.asarray(W_out, np.float32).copy()
    Wo[:, :KV] *= 0.5        # h2 stored as 2*h2
    bo = np.asarray(b_out, np.float32)

    # shared (same on all cores)
    emb_ext = np.zeros((EMB + 1, VOC + 1), np.float32)
    emb_ext[:EMB, :VOC] = emb.T
    emb_ext[EMB, VOC] = 1.0
    w1e_rhs = np.concatenate([W1[:, :EMB].T, b1[None, :]], axis=0)  # (257,2048)

    # w1_stat[k, c, g, m]: c=0 ctx (W1 cols 256:384), c=1..4 h1 (384:896)
    w1ct = W1[:, EMB:EMB + VS].T.reshape(P, 16, P)             # [k, g, m]
    w1h = W1[:, EMB + VS:].T.reshape(4, P, 16, P)              # [c, k, g, m]
    w1_stat = np.concatenate([w1ct[:, None], w1h.transpose(1, 0, 2, 3)],
                             axis=1).astype(BF16_NP)           # (128,5,16,128)

    w2h2 = W2[:, HID:].T.reshape(P, 4, P)                      # [k, g, m]
    w2h1 = W2[:, :HID].T.reshape(4, P, 4, P)                   # [c, k, g, m]
    w2_stat = np.concatenate([w2h2[:, None], w2h1.transpose(1, 0, 2, 3)],
                             axis=1).astype(BF16_NP)           # (128,5,4,128)
    b2all = b2.reshape(4, P).astype(BF16_NP)
    sel4 = np.zeros((4, 4, NL), np.float32)
    for g in range(4):
        sel4[g, g, :] = 1.0
    sel4 = sel4.astype(BF16_NP)

    wout_stat = np.stack([Wo[:, :KV].T, Wo[:, KV:].T], axis=1).astype(BF16_NP)
    bout_col = bo.reshape(VOC, 1).astype(np.float32)

    tri = MASK_NEG * (np.arange(P)[None, :] >= np.arange(P)[:, None])
    tri_stat = tri.astype(BF16_NP)                             # [k, t]

    shared = dict(emb_ext=emb_ext, w1e_rhs=w1e_rhs, w1_stat=w1_stat,
                  w2_stat=w2_stat, b2all=b2all, sel4=sel4,
                  wout_stat=wout_stat, bout_col=bout_col, tri_stat=tri_stat)

    ranks, cnt, plan = make_plan(lens)
    NCH = len(plan)
    in_maps = []
    key_h = key * 0.5        # energy uses h2 stored as 2*h2
    for c in range(NCORES):
        B = ranks[c::NCORES]
        kh = key_h[:, B, :]                                    # (T, 16, KV)
        vv = values[:, B, :]                                   # (T, 16, VS)
        lensB = lens[B]
        # packed stationaries: key_pack[k, ci, t], val_pack[t, ci, v]
        key_pack = np.zeros((KV, NCH, P), np.float32)
        val_pack = np.zeros((P, NCH, VS), np.float32)
        # thr masks, step-pattern per (slot, chunk) column
        thr = np.clip(lensB[:, None] - 128 * np.arange(TC)[None, :], 0, P)
        th_oh = np.zeros((P + 1, NL, TC), np.float32)
        nn2, tc2 = np.meshgrid(np.arange(NL), np.arange(TC), indexing="ij")
        th_oh[thr.reshape(-1), nn2.reshape(-1), tc2.reshape(-1)] = 1.0

        def tail_rows(s):
            return int(np.clip(lensB[s] - 128 * (cnt[s] - 1), 0, P))

        for ci, e in enumerate(plan):
            if e[0] == 's':
                _, s, cc = e
                key_pack[:, ci, :] = kh[128 * cc:128 * (cc + 1), s, :].T
                val_pack[:, ci, :] = vv[128 * cc:128 * (cc + 1), s, :]
            else:
                _, a, ca, b, cb = e
                ra, rb = tail_rows(a), tail_rows(b)
                if ra:
                    key_pack[:, ci, :ra] = kh[128 * ca:128 * ca + ra, a, :].T
                    val_pack[:ra, ci, :] = vv[128 * ca:128 * ca + ra, a, :]
                if rb:
                    key_pack[:, ci, ra:ra + rb] = \
                        kh[128 * cb:128 * cb + rb, b, :].T
                    val_pack[ra:ra + rb, ci, :] = \
                        vv[128 * cb:128 * cb + rb, b, :]
                # col a: mask t >= ra (one-hot at ra, as already built)
                # col b: mask t < ra and t >= ra+rb: steps +1@0 -1@ra +1@ra+rb
                vec = np.zeros(P + 1, np.float32)
                vec[0] += 1.0
                vec[min(ra, P)] -= 1.0
                vec[min(ra + rb, P)] += 1.0
                th_oh[:P, b, cb] = vec[:P]
                th_oh[P, b, cb] = 0.0
                vec2 = np.zeros(P + 1, np.float32)
                vec2[min(ra, P)] += 1.0
                th_oh[:P, a, ca] = vec2[:P]
                th_oh[P, a, ca] = 0.0
        th_oh = th_oh[:P].reshape(P, NL * TC)

        oh = np.zeros((VOC + 1, L, NL), np.float32)
        txt = text[B, :L]                                      # (16, L)
        nn_idx, tt_idx = np.meshgrid(np.arange(NL), np.arange(L),
                                     indexing="ij")
        oh[txt.reshape(-1), tt_idx.reshape(-1), nn_idx.reshape(-1)] = 1.0
        oh[VOC, :, :] = 1.0
        oh3 = np.zeros((P, L, 2, NL), np.float32)              # block-diag
        for j in range(2):
            oh3[j * 64:j * 64 + VOC + 1, :, j, :] = oh
        in_maps.append(dict(shared,
                            key_stat=key_pack.astype(BF16_NP),
                            val_stat=val_pack.astype(BF16_NP),
                            onehot3=oh3.astype(BF16_NP),
                            thr_oh=th_oh.astype(BF16_NP)))
    return in_maps


_MODULE_CACHE = {}


def kernel(**inputs):
    L = int(np.asarray(inputs["text"]).shape[1])
    ranks, cnt, plan = make_plan(inputs["lens"])
    key_c = (L, cnt, plan)
    if key_c not in _MODULE_CACHE:
        _MODULE_CACHE[key_c] = build_module(L, cnt=cnt, plan=plan)
    nc = _MODULE_CACHE[key_c]
    in_maps = make_in_maps(**inputs, L=L)
    res = run_bass_kernel_spmd(nc, in_maps, core_ids=list(range(NCORES)))
    out = np.zeros((NB, L, VOC), np.float32)
    for c in range(NCORES):
        o = res.results[c]["out"]                              # (34, L, 16)
        for s in range(NL):
            out[ranks[s * NCORES + c]] = o[:, :, s].T
    return out



# revision 14
# speedup vs baseline: 2.0765x; 1.2700x over previous
"""Trainium2 Bass kernel for the attention LSTM decoder (nn_Decoder).

Data-parallel over batch N=128 across 8 cores (NL=16 each); per core a
256-step teacher-forced decode. The per-step critical ring (PE gate
matmuls -> ACT tanh -> DVE fused pointwise -> attention -> softmax ->
context) is minimized:

  - LSTM pointwise via fused scalar_tensor_tensor ops (3 DVE ops/cell):
    cell buffer rows [t_i | t_f | t_g | S(=2c) | t_o] make every fusion a
    plain row slice:  uv = (cb[i,f]+1) * cb[g,S] -> [v, u];
    S' = 0.5u + v;  h' = (t_o+1)*tanh(S'/2)  (= 2h; consumer weight
    columns pre-halved, g-gate rows pre-doubled for the tanh trick).
    The o-gate tanh runs off the critical path.
  - softmax mask folded into the energy matmul: one constant triangular
    stationary x threshold-one-hot moving writes the -30000 step masks
    for all (slot, chunk) columns, including skipped chunks.
  - softmax denominators via 4 accumulating all-ones matmuls, produced
    already broadcast across partitions: one DVE reciprocal + one
    multiply, no gpsimd round trip.
  - PRE (embedding+bias) matmuls 2-packed at 64-row alignment (8 MMs),
    L2 bias via one select matmul, head bias folded into the ACT copy
    that stages 8 steps of logits per PSUM drain.
  - cross-step interleave: W_hh1/PRE matmuls of step t+2 are emitted
    under the pointwise chains of step t+1 to keep the PE fed; ragged
    encoder lengths prune attention chunks per slot (batch dealt to
    slots by sorted length so all cores share one SPMD program).
"""

import contextlib

import numpy as np
import ml_dtypes

import concourse.bacc as bacc
import concourse.bass as bass
import concourse.mybir as mybir
import concourse.tile as tile
from concourse.bass_utils import run_bass_kernel_spmd

T_ENC = 512
NB = 128
L_FULL = 256
KV = 128
VS = 128
EMB = 256
HID = 512
VOC = 34
NCORES = 8
NL = NB // NCORES          # batch per core = 16
TC = T_ENC // 128          # 4 t-chunks
P = 128

F32 = mybir.dt.float32
BF16 = mybir.dt.bfloat16
AF = mybir.ActivationFunctionType
OP = mybir.AluOpType
AX = mybir.AxisListType
BF16_NP = ml_dtypes.bfloat16

MASK_NEG = -30000.0
HEAD_BATCH = 8


def _gate_perm(h):
    # keep pytorch gate order [i, f, g, o]
    return np.arange(4 * h)


def _kernel_body(nc, tcx, io, L, reps=1, cnt=(TC,) * NL, plan=None):
    if plan is None:
        plan = tuple(('s', s, c) for s in range(NL) for c in range(cnt[s]))
    NCH = len(plan)
    with contextlib.ExitStack() as stack:
        const = stack.enter_context(tcx.tile_pool(name="const", bufs=1))
        work = stack.enter_context(tcx.tile_pool(name="work", bufs=2))

        # ---- constant SBUF tensors (loaded once) ----
        key_stat = const.tile([P, NCH, P], BF16, tag="key_stat")
        val_stat = const.tile([P, NCH, P], BF16, tag="val_stat")
        w1_stat = const.tile([P, 5, 16, P], BF16, tag="w1_stat")
        w2_stat = const.tile([P, 5, 4, P], BF16, tag="w2_stat")
        wout_stat = const.tile([P, 2, VOC], BF16, tag="wout_stat")
        b2all = const.tile([4, P], BF16, tag="b2all")
        sel4 = const.tile([4, 4, NL], BF16, tag="sel4")
        bout_col = const.tile([VOC, 1], F32, tag="bout_col")
        onehot3 = const.tile([P, L, 2, NL], BF16, tag="onehot3")
        pre_stat = const.tile([P, 8, P], BF16, tag="pre_stat")
        staging = const.tile([VOC, L, NL], F32, tag="staging")

        ones128 = const.tile([P, P], BF16, tag="ones128")
        tri_stat = const.tile([P, P], BF16, tag="tri_stat")
        thr_oh = const.tile([P, NL * TC], BF16, tag="thr_oh")

        # persistent cell buffers, rows [t_i | t_f | t_g | S(=2c) | t_o]:
        # the gate tanh lands around the S rows so the fused STTs below use
        # plain row slices: uv = (cb[i,f]+1) * cb[g,S] -> [v, u]
        cb1 = const.tile([P, 20, NL], F32, tag="cb1")
        cb2 = const.tile([P, 5, NL], F32, tag="cb2")
        h1d = const.tile([P, 4, NL], BF16, tag="h1d")
        h2d = const.tile([P, NL], BF16, tag="h2d")
        ctxT = const.tile([P, NL], BF16, tag="ctxT")

        # ---- prologue: DMA inputs ----
        nc.sync.dma_start(out=key_stat, in_=io["key_stat"].ap())
        nc.sync.dma_start(out=val_stat, in_=io["val_stat"].ap())
        nc.sync.dma_start(out=w1_stat, in_=io["w1_stat"].ap())
        nc.sync.dma_start(out=w2_stat, in_=io["w2_stat"].ap())
        nc.sync.dma_start(out=wout_stat, in_=io["wout_stat"].ap())
        nc.sync.dma_start(out=b2all, in_=io["b2all"].ap())
        nc.sync.dma_start(out=sel4, in_=io["sel4"].ap())
        nc.sync.dma_start(out=bout_col, in_=io["bout_col"].ap())
        nc.sync.dma_start(out=onehot3, in_=io["onehot3"].ap())
        nc.sync.dma_start(out=tri_stat, in_=io["tri_stat"].ap())
        nc.sync.dma_start(out=thr_oh, in_=io["thr_oh"].ap())

        nc.vector.memset(ones128, 1.0)
        nc.vector.memset(cb1, 0.0)
        nc.vector.memset(cb2, 0.0)
        nc.vector.memset(h1d, 0.0)
        nc.vector.memset(h2d, 0.0)
        nc.vector.memset(ctxT, 0.0)

        # ---- prologue: PRE = emb_ext.T @ w1e_rhs  -> (35, 2048) bf16 ----
        with tcx.tile_pool(name="prep", bufs=1) as prep, \
             tcx.tile_pool(name="prepp", bufs=1, space="PSUM") as prepp:
            emb_a = prep.tile([P, VOC + 1], F32, tag="emb_a")
            emb_b = prep.tile([P, VOC + 1], F32, tag="emb_b")
            emb_c = prep.tile([1, VOC + 1], F32, tag="emb_c")
            rhs_a = prep.tile([P, 2048], F32, tag="rhs_a")
            rhs_b = prep.tile([P, 2048], F32, tag="rhs_b")
            rhs_c = prep.tile([1, 2048], F32, tag="rhs_c")
            nc.sync.dma_start(out=emb_a, in_=io["emb_ext"].ap()[0:P, :])
            nc.sync.dma_start(out=emb_b, in_=io["emb_ext"].ap()[P:2 * P, :])
            nc.sync.dma_start(out=emb_c, in_=io["emb_ext"].ap()[2 * P:2 * P + 1, :])
            nc.sync.dma_start(out=rhs_a, in_=io["w1e_rhs"].ap()[0:P, :])
            nc.sync.dma_start(out=rhs_b, in_=io["w1e_rhs"].ap()[P:2 * P, :])
            nc.sync.dma_start(out=rhs_c, in_=io["w1e_rhs"].ap()[2 * P:2 * P + 1, :])
            nc.vector.memset(pre_stat, 0.0)
            for nf in range(4):
                pp = prepp.tile([VOC + 1, 512], F32, tag="prepsum")
                sl = slice(nf * 512, (nf + 1) * 512)
                nc.tensor.matmul(pp, emb_a, rhs_a[:, sl], start=True, stop=False)
                nc.tensor.matmul(pp, emb_b, rhs_b[:, sl], start=False, stop=False)
                nc.tensor.matmul(pp, emb_c, rhs_c[:, sl], start=False, stop=True)
                # gtile gt = nf*4 + j lands at pre_stat[64*(gt%2), gt//2, :]
                for j in range(4):
                    gt = nf * 4 + j
                    blk, row = gt // 2, gt % 2
                    nc.scalar.copy(
                        out=pre_stat[row * 64:row * 64 + VOC + 1, blk, :],
                        in_=pp[:, j * P:(j + 1) * P])

        pg1 = stack.enter_context(tcx.tile_pool(name="pg1", bufs=2, space="PSUM"))
        pg2 = stack.enter_context(tcx.tile_pool(name="pg2", bufs=1, space="PSUM"))
        pE = stack.enter_context(tcx.tile_pool(name="pE", bufs=1, space="PSUM"))
        psm = stack.enter_context(tcx.tile_pool(name="psm", bufs=1, space="PSUM"))
        pcx = stack.enter_context(tcx.tile_pool(name="pcx", bufs=1, space="PSUM"))
        ppd = stack.enter_context(tcx.tile_pool(name="ppd", bufs=1, space="PSUM"))
        pht = stack.enter_context(tcx.tile_pool(name="pht", bufs=1, space="PSUM"))

        att_list = [(s, c) for s in range(NL) for c in range(cnt[s])]

        ABL = set(__import__("os").environ.get("ABL", "").split(","))
        HEAT = int(__import__("os").environ.get("PE_HEAT", "0"))
        heat_tile = (pht.tile([P, P], F32, tag="heat", name="heat")
                     if HEAT else None)

        def heater(k):
            # dependency-free matmuls keep the PE p-state warm during the
            # pointwise chains (hardware re-throttles an idle PE)
            for _ in range(k):
                nc.tensor.matmul(heat_tile[:, 0:P], ones128,
                                 key_stat[:, 0, 0, :],
                                 start=True, stop=True, skip_group_check=True)
        ep = pE.tile([P, NL * TC], F32, tag="ep")

        g1ps = {}
        g2ps = {}
        pps = {}

        def emit_pre(t):
            g1p = pg1.tile([P, 16, NL], F32, tag="g1p", name="g1p")
            g1ps[t] = g1p
            for b in range(8):
                nc.tensor.matmul(g1p[:, 2 * b:2 * b + 2, :],
                                 pre_stat[:, b, :],
                                 onehot3[:, t, :, :],
                                 start=(b == 0), stop=False)

        def emit_h1mms(t):
            if "noh1" in ABL:
                return
            g1p = g1ps[t]
            for c in range(4):
                for g in range(16):
                    nc.tensor.matmul(g1p[:, g, :], w1_stat[:, 1 + c, g, :],
                                     h1d[:, c, :], start=False, stop=False)

        def emit_ctx_mms(t):
            g1p = g1ps[t]
            for g in ([15] if "noctx" in ABL else range(16)):
                nc.tensor.matmul(g1p[:, g, :], w1_stat[:, 0, g, :], ctxT,
                                 start=False, stop=(g == 15))

        def emit_l1_pointwise(t):
            g1p = g1ps.pop(t)
            if "nopw" in ABL:
                return
            # i,f,g tanh first (feeds uv); o tanh runs off the critical path
            nc.scalar.activation(cb1[:, 0:12, :], g1p[:, 0:12, :],
                                 AF.Tanh, scale=0.5)
            nc.scalar.activation(cb1[:, 16:20, :], g1p[:, 12:16, :],
                                 AF.Tanh, scale=0.5)
            uv1 = work.tile([P, 8, NL], F32, tag="uv1")
            th1 = work.tile([P, 4, NL], F32, tag="th1")
            # [v, u] = (cb[i, f] + 1) * cb[g, S]
            nc.vector.scalar_tensor_tensor(uv1, cb1[:, 0:8, :], 1.0,
                                           cb1[:, 8:16, :],
                                           op0=OP.add, op1=OP.mult)
            # S' = 0.5*u + v
            nc.vector.scalar_tensor_tensor(cb1[:, 12:16, :], uv1[:, 4:8, :],
                                           0.5, uv1[:, 0:4, :],
                                           op0=OP.mult, op1=OP.add)
            nc.scalar.activation(th1, cb1[:, 12:16, :], AF.Tanh, scale=0.5)
            nc.vector.scalar_tensor_tensor(h1d, cb1[:, 16:20, :], 1.0, th1,
                                           op0=OP.add, op1=OP.mult)

        def emit_g2(t):
            g2p = pg2.tile([P, 4, NL], F32, tag="g2p", name="g2p")
            g2ps[t] = g2p
            nc.tensor.matmul(g2p, b2all, sel4, start=True, stop=False)
            if "nog2" in ABL:
                nc.tensor.matmul(g2p[:, 3, :], w2_stat[:, 4, 3, :],
                                 h1d[:, 3, :], start=False, stop=True)
                return
            for g in range(4):
                nc.tensor.matmul(g2p[:, g, :], w2_stat[:, 0, g, :], h2d,
                                 start=False, stop=False)
            for c in range(4):
                for g in range(4):
                    nc.tensor.matmul(g2p[:, g, :], w2_stat[:, 1 + c, g, :],
                                     h1d[:, c, :],
                                     start=False, stop=(c == 3 and g == 3))

        def emit_l2_pointwise(t):
            g2p = g2ps.pop(t)
            if "nopw" in ABL:
                return
            nc.scalar.activation(cb2[:, 0:3, :], g2p[:, 0:3, :],
                                 AF.Tanh, scale=0.5)
            nc.scalar.activation(cb2[:, 4:5, :], g2p[:, 3:4, :],
                                 AF.Tanh, scale=0.5)
            uv2 = work.tile([P, 2, NL], F32, tag="uv2")
            th2 = work.tile([P, 1, NL], F32, tag="th2")
            nc.vector.scalar_tensor_tensor(uv2, cb2[:, 0:2, :], 1.0,
                                           cb2[:, 2:4, :],
                                           op0=OP.add, op1=OP.mult)
            nc.vector.scalar_tensor_tensor(cb2[:, 3:4, :], uv2[:, 1:2, :],
                                           0.5, uv2[:, 0:1, :],
                                           op0=OP.mult, op1=OP.add)
            nc.scalar.activation(th2, cb2[:, 3:4, :], AF.Tanh, scale=0.5)
            nc.vector.scalar_tensor_tensor(h2d, cb2[:, 4, :], 1.0,
                                           th2[:, 0, :],
                                           op0=OP.add, op1=OP.mult)

        HB = NL // 2

        def _att_aps(e, buf_pn, buf_ch):
            # (moving/out) APs for a plan entry against a [P, NL] tile
            # (buf_pn: h2d or cxp) and a flat [P, NL*TC] tile (buf_ch: ep
            # or expe)
            if e[0] == 's':
                _, s, c = e
                return buf_pn[:, s:s + 1], buf_ch[:, s * TC + c:s * TC + c + 1]
            _, a, ca, b, cb = e
            ia, ib = a * TC + ca, b * TC + cb
            return (buf_pn[:, a:b + 1:b - a] if b - a > 1 else buf_pn[:, a:b + 1],
                    buf_ch[:, ia:ib + 1:ib - ia])

        PAIRMODE = __import__("os").environ.get("PAIRMODE", "both")

        def _att_sub(e):
            # split a pair entry into two single-col (mov, out) sub-entries
            # against the SAME merged stationary chunk
            _, a, ca, b, cb = e
            return [(a, a * TC + ca), (b, b * TC + cb)]

        def emit_attention(t):
            # mask via one matmul: tri_stat[k, t] = MASK_NEG * [t >= k],
            # thr_oh[k, col] step-pattern masking invalid rows of each col
            nc.tensor.matmul(ep, tri_stat, thr_oh, start=True, stop=False)
            att_l = plan[-1:] if "noatt" in ABL else plan
            esplit = PAIRMODE in ("esplit", "split")
            for i, e in enumerate(att_l):
                ci = plan.index(e) if "noatt" in ABL else i
                last = (i == len(att_l) - 1)
                if e[0] == 'p' and esplit:
                    for k, (s, col) in enumerate(_att_sub(e)):
                        nc.tensor.matmul(ep[:, col:col + 1],
                                         key_stat[:, ci, :],
                                         h2d[:, s:s + 1],
                                         start=False,
                                         stop=(last and k == 1))
                    continue
                mov, out = _att_aps(e, h2d, ep)
                nc.tensor.matmul(out, key_stat[:, ci, :], mov,
                                 start=False, stop=last)
            expe = work.tile([P, NL * TC], BF16, tag="expe")
            nc.scalar.activation(expe, ep, AF.Exp)

            # denominators, already broadcast across partitions: accumulate
            # ones128.T @ expe[:, :, tc] over the 4 chunks
            sums_b = psm.tile([P, NL], F32, tag="sums_b")
            sum_l = range(1) if "noatt" in ABL else range(TC)
            for tcc in sum_l:
                nc.tensor.matmul(sums_b, ones128, expe[:, tcc::TC],
                                 start=(tcc == 0), stop=(tcc == len(sum_l) - 1))
            cxp = pcx.tile([P, NL], F32, tag="cxp")
            cx_l = plan[-2:] if "noatt" in ABL else plan
            csplit = PAIRMODE in ("csplit", "split")
            for i, e in enumerate(cx_l):
                ci = plan.index(e) if "noatt" in ABL else i
                first, last = (i == 0), (i == len(cx_l) - 1)
                if e[0] == 'p' and csplit:
                    for k, (s, col) in enumerate(_att_sub(e)):
                        nc.tensor.matmul(cxp[:, s:s + 1],
                                         val_stat[:, ci, :],
                                         expe[:, col:col + 1],
                                         start=(first and k == 0),
                                         stop=(last and k == 1))
                    continue
                mov, out = _att_aps(e, cxp, expe)
                nc.tensor.matmul(mov, val_stat[:, ci, :], out,
                                 start=first, stop=last)
            recip_b = work.tile([P, NL], F32, tag="recip_b")
            nc.vector.reciprocal(out=recip_b, in_=sums_b)
            nc.vector.tensor_mul(ctxT, cxp, recip_b)

        def emit_head(t):
            slot = t % HEAD_BATCH
            if slot == 0:
                pps[0] = ppd.tile([VOC, HEAD_BATCH, NL], F32, tag="pp", name="pp")
            pp = pps[0]
            nc.tensor.matmul(pp[:, slot, :], wout_stat[:, 0, :], h2d,
                             start=True, stop=False)
            nc.tensor.matmul(pp[:, slot, :], wout_stat[:, 1, :], ctxT,
                             start=False, stop=True)
            if slot == HEAD_BATCH - 1:
                nc.scalar.add(staging[:, t - slot:t + 1, :], pp, bout_col)

        rep_ctx = tcx.For_i(0, reps, 1) if reps > 1 else contextlib.nullcontext()
        with rep_ctx:
            # LSTM step 0 (ctx/h start at zero)
            emit_pre(0)
            emit_h1mms(0)
            emit_ctx_mms(0)
            emit_l1_pointwise(0)
            if L > 1:
                emit_pre(1)
            emit_g2(0)
            if L > 1:
                emit_h1mms(1)
            emit_l2_pointwise(0)

            for t in range(L):
                emit_attention(t)
                if HEAT:
                    heater(1)
                if t + 1 < L:
                    emit_ctx_mms(t + 1)
                emit_head(t)
                if t + 1 < L:
                    emit_l1_pointwise(t + 1)
                    if t + 2 < L:
                        emit_pre(t + 2)
                    if HEAT:
                        heater(1)
                    emit_g2(t + 1)
                    if t + 2 < L:
                        emit_h1mms(t + 2)
                    if HEAT:
                        heater(1)
                    emit_l2_pointwise(t + 1)

            if L % HEAD_BATCH:
                rem = L % HEAD_BATCH
                nc.scalar.copy(out=staging[:, L - rem:L, :],
                               in_=pps[0][:, 0:rem, :])

        # ---- epilogue ----
        nc.sync.dma_start(out=io["out"].ap(), in_=staging)


def build_module(L=L_FULL, reps=1, cnt=(TC,) * NL, plan=None):
    if plan is None:
        plan = tuple(('s', s, c) for s in range(NL) for c in range(cnt[s]))
    NCH = len(plan)
    nc = bacc.Bacc("TRN2", target_bir_lowering=False, debug=False)
    io = {}
    io["key_stat"] = nc.dram_tensor("key_stat", [P, NCH, P], BF16,
                                    kind="ExternalInput")
    io["val_stat"] = nc.dram_tensor("val_stat", [P, NCH, P], BF16,
                                    kind="ExternalInput")
    io["w1_stat"] = nc.dram_tensor("w1_stat", [P, 5, 16, P], BF16,
                                   kind="ExternalInput")
    io["w2_stat"] = nc.dram_tensor("w2_stat", [P, 5, 4, P], BF16,
                                   kind="ExternalInput")
    io["wout_stat"] = nc.dram_tensor("wout_stat", [P, 2, VOC], BF16,
                                     kind="ExternalInput")
    io["b2all"] = nc.dram_tensor("b2all", [4, P], BF16,
                                 kind="ExternalInput")
    io["sel4"] = nc.dram_tensor("sel4", [4, 4, NL], BF16,
                                kind="ExternalInput")
    io["bout_col"] = nc.dram_tensor("bout_col", [VOC, 1], F32,
                                    kind="ExternalInput")
    io["onehot3"] = nc.dram_tensor("onehot3", [P, L, 2, NL],
                                   BF16, kind="ExternalInput")
    io["tri_stat"] = nc.dram_tensor("tri_stat", [P, P], BF16,
                                    kind="ExternalInput")
    io["thr_oh"] = nc.dram_tensor("thr_oh", [P, NL * TC], BF16,
                                  kind="ExternalInput")
    io["emb_ext"] = nc.dram_tensor("emb_ext", [EMB + 1, VOC + 1], F32,
                                   kind="ExternalInput")
    io["w1e_rhs"] = nc.dram_tensor("w1e_rhs", [EMB + 1, 4 * HID], F32,
                                   kind="ExternalInput")
    io["out"] = nc.dram_tensor("out", [VOC, L, NL], F32,
                               kind="ExternalOutput")

    with tile.TileContext(nc) as tcx:
        _kernel_body(nc, tcx, io, L, reps=reps, cnt=cnt, plan=plan)
    nc.compile()
    return nc


def batch_assignment(lens):
    """Deal batch elements (sorted by length desc) round-robin to cores."""
    lens = np.asarray(lens).astype(np.int64)
    ranks = np.argsort(-lens, kind="stable")
    cnt = tuple(int(-(-lens[ranks[s * NCORES]] // 128)) for s in range(NL))
    return ranks, cnt


def make_plan(lens):
    """Attention chunk plan shared by all cores (SPMD template).

    Entries: ('s', s, c) one slot-chunk per stationary, or ('p', a, ca, b, cb)
    merging the partial tail chunks of slots a and b (a < b) into one
    stationary (softmax is permutation-invariant over t, so each slot's
    valid timesteps can be re-packed; per-core row offsets and masks are
    data, not program).
    """
    lens = np.asarray(lens).astype(np.int64)
    ranks, cnt = batch_assignment(lens)
    ra_max = []
    for s in range(NL):
        rs = [int(lens[ranks[s * NCORES + c]]) - 128 * (cnt[s] - 1)
              for c in range(NCORES)]
        ra_max.append(max(0, max(rs)))
    order = sorted(range(NL), key=lambda s: -ra_max[s])
    pairs, singles = [], []
    i, j = 0, NL - 1
    if __import__("os").environ.get("NOPAIR"):
        i = NL
        singles = list(range(NL))
    while i < j:
        a, b = order[i], order[j]
        if ra_max[a] + ra_max[b] <= P:
            pairs.append((a, b))
            i += 1
            j -= 1
        else:
            singles.append(a)
            i += 1
    if i == j:
        singles.append(order[i])
    plan = []
    for s in range(NL):
        for c in range(cnt[s] - 1):
            plan.append(('s', s, c))
    for s in sorted(singles):
        plan.append(('s', s, cnt[s] - 1))
    for a, b in pairs:
        a, b = min(a, b), max(a, b)
        plan.append(('p', a, cnt[a] - 1, b, cnt[b] - 1))
    return ranks, cnt, tuple(plan)


def make_in_maps(key, values, lens, text, emb, W_ih1, W_hh1, b_ih1, b_hh1,
                 W_ih2, W_hh2, b_ih2, b_hh2, W_out, b_out, L=L_FULL):
    key = np.asarray(key, np.float32)
    values = np.asarray(values, np.float32)
    lens = np.asarray(lens).astype(np.int64)
    text = np.asarray(text).astype(np.int64)
    emb = np.asarray(emb, np.float32)

    perm1 = _gate_perm(HID)
    W1 = np.concatenate([np.asarray(W_ih1, np.float32),
                         np.asarray(W_hh1, np.float32)], axis=1)[perm1]
    b1 = (np.asarray(b_ih1, np.float32) + np.asarray(b_hh1, np.float32))[perm1]
    W1 = W1.copy()
    b1 = b1.copy()
    W1[2 * HID:3 * HID] *= 2.0   # g-gate rows x2 (tanh trick)
    b1[2 * HID:3 * HID] *= 2.0
    W1[:, EMB + VS:] *= 0.5  # h1 stored as 2*h1

    perm2 = _gate_perm(KV)
    W2 = np.concatenate([np.asarray(W_ih2, np.float32),
                         np.asarray(W_hh2, np.float32)], axis=1)[perm2]
    b2 = (np.asarray(b_ih2, np.float32) + np.asarray(b_hh2, np.float32))[perm2]
    W2 = W2.copy()
    b2 = b2.copy()
    W2[2 * KV:3 * KV] *= 2.0
    b2[2 * KV:3 * KV] *= 2.0
    W2 *= 0.5                # both h1 and h2 inputs carry a 2x factor

    Wo = np.asarray(W_out, np.float32).copy()
    Wo[:, :KV] *= 0.5        # h2 stored as 2*h2
    bo = np.asarray(b_out, np.float32)

    # shared (same on all cores)
    emb_ext = np.zeros((EMB + 1, VOC + 1), np.float32)
    emb_ext[:EMB, :VOC] = emb.T
    emb_ext[EMB, VOC] = 1.0
    w1e_rhs = np.concatenate([W1[:, :EMB].T, b1[None, :]], axis=0)  # (257,2048)

    # w1_stat[k, c, g, m]: c=0 ctx (W1 cols 256:384), c=1..4 h1 (384:896)
    w1ct = W1[:, EMB:EMB + VS].T.reshape(P, 16, P)             # [k, g, m]
    w1h = W1[:, EMB + VS:].T.reshape(4, P, 16, P)              # [c, k, g, m]
    w1_stat = np.concatenate([w1ct[:, None], w1h.transpose(1, 0, 2, 3)],
                             axis=1).astype(BF16_NP)           # (128,5,16,128)

    w2h2 = W2[:, HID:].T.reshape(P, 4, P)                      # [k, g, m]
    w2h1 = W2[:, :HID].T.reshape(4, P, 4, P)                   # [c, k, g, m]
    w2_stat = np.concatenate([w2h2[:, None], w2h1.transpose(1, 0, 2, 3)],
                             axis=1).astype(BF16_NP)           # (128,5,4,128)
    b2all = b2.reshape(4, P).astype(BF16_NP)
    sel4 = np.zeros((4, 4, NL), np.float32)
    for g in range(4):
        sel4[g, g, :] = 1.0
    sel4 = sel4.astype(BF16_NP)

    wout_stat = np.stack([Wo[:, :KV].T, Wo[:, KV:].T], axis=1).astype(BF16_NP)
    bout_col = bo.reshape(VOC, 1).astype(np.float32)

    tri = MASK_NEG * (np.arange(P)[None, :] >= np.arange(P)[:, None])
    tri_stat = tri.astype(BF16_NP)                             # [k, t]

    shared = dict(emb_ext=emb_ext, w1e_rhs=w1e_rhs, w1_stat=w1_stat,
                  w2_stat=w2_stat, b2all=b2all, sel4=sel4,
                  wout_stat=wout_stat, bout_col=bout_col, tri_stat=tri_stat)

    ranks, cnt, plan = make_plan(lens)
    NCH = len(plan)
    in_maps = []
    key_h = key * 0.5        # energy uses h2 stored as 2*h2
    for c in range(NCORES):
        B = ranks[c::NCORES]
        kh = key_h[:, B, :]                                    # (T, 16, KV)
        vv = values[:, B, :]                                   # (T, 16, VS)
        lensB = lens[B]
        # packed stationaries: key_pack[k, ci, t], val_pack[t, ci, v]
        key_pack = np.zeros((KV, NCH, P), np.float32)
        val_pack = np.zeros((P, NCH, VS), np.float32)
        # thr masks, step-pattern per (slot, chunk) column
        thr = np.clip(lensB[:, None] - 128 * np.arange(TC)[None, :], 0, P)
        th_oh = np.zeros((P + 1, NL, TC), np.float32)
        nn2, tc2 = np.meshgrid(np.arange(NL), np.arange(TC), indexing="ij")
        th_oh[thr.reshape(-1), nn2.reshape(-1), tc2.reshape(-1)] = 1.0

        def tail_rows(s):
            return int(np.clip(lensB[s] - 128 * (cnt[s] - 1), 0, P))

        for ci, e in enumerate(plan):
            if e[0] == 's':
                _, s, cc = e
                key_pack[:, ci, :] = kh[128 * cc:128 * (cc + 1), s, :].T
                val_pack[:, ci, :] = vv[128 * cc:128 * (cc + 1), s, :]
            else:
                _, a, ca, b, cb = e
                ra, rb = tail_rows(a), tail_rows(b)
                if ra:
                    key_pack[:, ci, :ra] = kh[128 * ca:128 * ca + ra, a, :].T
                    val_pack[:ra, ci, :] = vv[128 * ca:128 * ca + ra, a, :]
                if rb:
                    key_pack[:, ci, ra:ra + rb] = \
                        kh[128 * cb:128 * cb + rb, b, :].T
                    val_pack[ra:ra + rb, ci, :] = \
                        vv[128 * cb:128 * cb + rb, b, :]
                # col a: mask t >= ra (one-hot at ra, as already built)
                # col b: mask t < ra and t >= ra+rb: steps +1@0 -1@ra +1@ra+rb
                vec = np.zeros(P + 1, np.float32)
                vec[0] += 1.0
                vec[min(ra, P)] -= 1.0
                vec[min(ra + rb, P)] += 1.0
                th_oh[:P, b, cb] = vec[:P]
                th_oh[P, b, cb] = 0.0
                vec2 = np.zeros(P + 1, np.float32)
                vec2[min(ra, P)] += 1.0
                th_oh[:P, a, ca] = vec2[:P]
                th_oh[P, a, ca] = 0.0
        th_oh = th_oh[:P].reshape(P, NL * TC)

        oh = np.zeros((VOC + 1, L, NL), np.float32)
        txt = text[B, :L]                                      # (16, L)
        nn_idx, tt_idx = np.meshgrid(np.arange(NL), np.arange(L),
                                     indexing="ij")
        oh[txt.reshape(-1), tt_idx.reshape(-1), nn_idx.reshape(-1)] = 1.0
        oh[VOC, :, :] = 1.0
        oh3 = np.zeros((P, L, 2, NL), np.float32)              # block-diag
        for j in range(2):
            oh3[j * 64:j * 64 + VOC + 1, :, j, :] = oh
        in_maps.append(dict(shared,
                            key_stat=key_pack.astype(BF16_NP),
                            val_stat=val_pack.astype(BF16_NP),
                            onehot3=oh3.astype(BF16_NP),
                            thr_oh=th_oh.astype(BF16_NP)))
    return in_maps


_MODULE_CACHE = {}


def kernel(**inputs):
    L = int(np.asarray(inputs["text"]).shape[1])
    ranks, cnt, plan = make_plan(inputs["lens"])
    key_c = (L, cnt, plan)
    if key_c not in _MODULE_CACHE:
        _MODULE_CACHE[key_c] = build_module(L, cnt=cnt, plan=plan)
    nc = _MODULE_CACHE[key_c]
    in_maps = make_in_maps(**inputs, L=L)
    res = run_bass_kernel_spmd(nc, in_maps, core_ids=list(range(NCORES)))
    out = np.zeros((NB, L, VOC), np.float32)
    for c in range(NCORES):
        o = res.results[c]["out"]                              # (34, L, 16)
        for s in range(NL):
            out[ranks[s * NCORES + c]] = o[:, :, s].T
    return out

